# revision 27
# baseline (speedup 1.0000x reference)
"""Trainium2 Bass kernel for nn_Recommender_2 (moe_routing).

Pipeline per core (8 NeuronCores, one TRN2 chip):
  Phase 1 (data-parallel over batch, 128 rows/core):
    - indirect-DMA gather of ratings embeddings (table pre-cast to bf16)
    - DMA-xbar transposes to time-major x^T tiles (no PE/DVE involvement)
    - 2-layer LSTM scan (T=50) in transposed [gate, batch] layout:
      weights stationary (lhsT), state h^T flows as rhs -> no per-step
      transposes. Gate order host-permuted to (g,i,f,o) so the cell needs
      only 3 ACT calls: tanh(g) after PSUM bank0, sigmoid(i|f|o) as one
      768-wide call, tanh(c). The two cell products t1=i*g / t2=f*c run
      on DVE and GpSimd in parallel (GpSimd once the embedding gathers
      have drained its queue). Layer-0 input MMs pipelined one step ahead
      into dedicated PSUM tiles; layer-1 runs one step delayed.
    - ratings MLP + user MLP (transposed layout) -> z^T [384,128] bf16
  Exchange: AllGather of z^T across the 8 cores -> Z^T [384,1024]
  Phase 2 (expert-parallel, 8 experts/core over full batch):
    - he = z @ W1_e accumulated in PSUM (bf16 weights)
    - the second GEMM (he @ w2_e, blocked by the relu) is eliminated; the
      reduction rotates over three engines by local slot (mod 3):
      ACT -> |w2| folded into W1, hidden units sign-partitioned
      [pos | pad | neg | pad] (unaligned boundary), two Relu accum_out
      calls (s+ - s-); DVE / GpSimd -> native W1, one
      scalar_tensor_tensor (he max 0)*w2 with accum_out.
Host reassembles [1024, 64] from per-core [1024, 8] outputs.
"""
import numpy as np
import ml_dtypes

import concourse.bacc as bacc
import concourse.bass as bass
import concourse.mybir as mybir
import concourse.tile as tile
from concourse.bass_utils import run_bass_kernel_spmd
from concourse.masks import make_identity

P = 128
NCORES = 8
B, T = 1024, 50
RV, RD, RH = 100000, 128, 256
R_OUT = 256
UV, UD, UDATA, U_OUT = 50000, 64, 32, 128
E, EIN, EH = 64, 384, 1536
EPC = E // NCORES
BL = B // NCORES
UIN = UD + UDATA
UH = 192
RHID = 512
NG = 8

F32 = mybir.dt.float32
BF16 = mybir.dt.bfloat16
FP8 = mybir.dt.float8e4
I32 = mybir.dt.int32
AF = mybir.ActivationFunctionType
ALU = mybir.AluOpType

# All experts reduce on DVE: relu(he)*w2 with accum_out, native hidden
# order (no sign-partition padding -> H2 == EH exactly). ACT stays free
# in phase 2; PE streaming is the binding engine.
N_W2 = EPC

_cache = {}


def _prep(inputs):
    f = lambda k: np.asarray(inputs[k], dtype=np.float32)
    bf = lambda a: np.ascontiguousarray(a, dtype=ml_dtypes.bfloat16)
    e4 = lambda a: np.ascontiguousarray(a, dtype=ml_dtypes.float8_e4m3)
    f32c = lambda a: np.ascontiguousarray(a, dtype=np.float32)

    # gate permutation i,f,g,o -> g,i,f,o  (bank0: g,i -> tanh(g) early,
    # then one sigmoid over i|f|o)
    perm = np.r_[512:768, 0:256, 256:512, 768:1024]

    shared = {}
    lstm_bias = False
    for l in range(2):
        wih = f(f"lstm_W_ih_{l}")[perm]
        whh = f(f"lstm_W_hh_{l}")[perm]
        bias = (f(f"lstm_b_ih_{l}") + f(f"lstm_b_hh_{l}"))[perm]
        shared[f"wih{l}"] = bf(wih.T)            # [in, 1024]
        shared[f"whh{l}"] = bf(whh.T)            # [256, 1024]
        shared[f"lb{l}"] = f32c(bias.reshape(NG, P).T)
        lstm_bias = lstm_bias or bool(np.any(bias))

    shared["rw1"] = bf(f("r_W1"))
    shared["rw2"] = bf(f("r_W2"))
    shared["uw1"] = bf(f("u_W1"))
    shared["uw2"] = bf(f("u_W2"))
    mlpb = np.zeros((P, 9), np.float32)
    mlpb[:, 0:4] = f("r_b1").reshape(4, P).T
    mlpb[:, 4:6] = f("r_b2").reshape(2, P).T
    ub1 = f("u_b1")
    mlpb[:, 6] = ub1[0:128]
    mlpb[0:64, 7] = ub1[128:192]
    mlpb[:, 8] = f("u_b2")
    mlp_bias = bool(np.any(mlpb))
    shared["mlpb"] = mlpb

    shared["remb"] = bf(f("ratings_emb"))        # bf16 table
    shared["uemb"] = bf(f("user_emb"))           # bf16 table

    w1 = f("exp_W1")                             # [64, 384, 1536]
    w2 = f("exp_W2").reshape(E, EH)              # [64, 1536]
    b1 = f("exp_b1")                             # [64, 1536]
    b2 = f("exp_b2").reshape(E)
    exp_b1_nz = bool(np.any(b1))

    ridx = np.asarray(inputs["ratings_tensor"]).astype(np.int32)
    uids = np.asarray(inputs["user_ids"]).astype(np.int32)
    udata = f("user_data")

    in_maps = []
    for c in range(NCORES):
        m = dict(shared)
        sl = slice(c * BL, (c + 1) * BL)
        m["ridx"] = np.ascontiguousarray(ridx[sl])
        m["uid"] = np.ascontiguousarray(uids[sl].reshape(BL, 1))
        m["udata"] = bf(udata[sl])
        es = slice(c * EPC, (c + 1) * EPC)
        m["expw"] = bf(w1[es])                   # [8, 384, 1536]
        m["w2r"] = f32c(np.broadcast_to(
            w2[es][:, None, :], (N_W2, P, EH)))
        m["b1p"] = bf(b1[es])                    # [8, 1536]
        b2bc = np.zeros((P, E), np.float32)
        for cc in range(NCORES):
            for e in range(EPC):
                b2bc[:, cc * EPC + e] = b2[c * EPC + e]
        m["b2bc"] = b2bc
        in_maps.append(m)

    bp = dict(lstm_bias=lstm_bias, mlp_bias=mlp_bias, exp_b1=exp_b1_nz,
              exp_b2=bool(np.any(b2)))
    return in_maps, bp


def _chunks(width):
    return [(s, min(s + 512, width)) for s in range(0, width, 512)]


def _build(bp, sim_single=False):

    nc = bacc.Bacc("TRN2", target_bir_lowering=False)
    d_ridx = nc.dram_tensor("ridx", [BL, T], I32, kind="ExternalInput")
    d_uid = nc.dram_tensor("uid", [BL, 1], I32, kind="ExternalInput")
    d_udata = nc.dram_tensor("udata", [BL, UDATA], BF16, kind="ExternalInput")
    d_remb = nc.dram_tensor("remb", [RV, RD], BF16, kind="ExternalInput")
    d_uemb = nc.dram_tensor("uemb", [UV, UD], BF16, kind="ExternalInput")
    d_wih = [nc.dram_tensor(f"wih{l}", [RD if l == 0 else RH, 4 * RH], BF16,
                            kind="ExternalInput") for l in range(2)]
    d_whh = [nc.dram_tensor(f"whh{l}", [RH, 4 * RH], BF16, kind="ExternalInput")
             for l in range(2)]
    d_lb = [nc.dram_tensor(f"lb{l}", [P, NG], F32, kind="ExternalInput")
            for l in range(2)]
    d_rw1 = nc.dram_tensor("rw1", [RH, RHID], BF16, kind="ExternalInput")
    d_rw2 = nc.dram_tensor("rw2", [RHID, R_OUT], BF16, kind="ExternalInput")
    d_uw1 = nc.dram_tensor("uw1", [UIN, UH], BF16, kind="ExternalInput")
    d_uw2 = nc.dram_tensor("uw2", [UH, U_OUT], BF16, kind="ExternalInput")
    d_mlpb = nc.dram_tensor("mlpb", [P, 9], F32, kind="ExternalInput")
    d_expw = nc.dram_tensor("expw", [EPC, EIN, EH], BF16, kind="ExternalInput")
    d_w2r = nc.dram_tensor("w2r", [N_W2, P, EH], F32, kind="ExternalInput")
    d_b1p = nc.dram_tensor("b1p", [EPC, EH], BF16, kind="ExternalInput")
    d_b2bc = nc.dram_tensor("b2bc", [P, E], F32, kind="ExternalInput")
    d_out = nc.dram_tensor("out", [B, EPC], F32, kind="ExternalOutput")

    with tile.TileContext(nc) as tc:
        with (
            tc.tile_pool(name="sb", bufs=1) as sb,
            tc.tile_pool(name="dr", bufs=1, space="DRAM") as dr,
        ):
            # ---- latency-critical loads first (indices feed the gathers) ----
            ridx_t = sb.tile([BL, T], I32)
            nc.sync.dma_start(out=ridx_t[:], in_=d_ridx[:])
            uid_t = sb.tile([BL, 1], I32)
            nc.sync.dma_start(out=uid_t[:], in_=d_uid[:])
            Uin = sb.tile([P, P], BF16)
            nc.gpsimd.indirect_dma_start(
                out=Uin[:, 0:UD], out_offset=None, in_=d_uemb[:],
                in_offset=bass.IndirectOffsetOnAxis(ap=uid_t[:, 0:1], axis=0))
            nc.sync.dma_start(out=Uin[:, UD:UIN], in_=d_udata[:])
            X = sb.tile([P, T, RD], BF16)
            for t in range(T):
                nc.gpsimd.indirect_dma_start(
                    out=X[:, t, :], out_offset=None, in_=d_remb[:],
                    in_offset=bass.IndirectOffsetOnAxis(ap=ridx_t[:, t:t + 1], axis=0))

            # ---- small static weights ----
            wih_t = []
            whh_t = []
            for l in range(2):
                kin = RD if l == 0 else RH
                wt = []
                for kc in range(kin // P):
                    tl = sb.tile([P, 4 * RH], BF16, tag=f"wih{l}_{kc}")
                    nc.sync.dma_start(out=tl[:], in_=d_wih[l][kc * P:(kc + 1) * P, :])
                    wt.append(tl)
                wih_t.append(wt)
                ht = []
                for kc in range(2):
                    tl = sb.tile([P, 4 * RH], BF16, tag=f"whh{l}_{kc}")
                    nc.sync.dma_start(out=tl[:], in_=d_whh[l][kc * P:(kc + 1) * P, :])
                    ht.append(tl)
                whh_t.append(ht)
            lb_t = []
            for l in range(2):
                tl = sb.tile([P, NG], F32, tag=f"lb{l}")
                nc.sync.dma_start(out=tl[:], in_=d_lb[l][:])
                lb_t.append(tl)
            rw1_t = []
            for kc in range(2):
                tl = sb.tile([P, RHID], BF16, tag=f"rw1_{kc}")
                nc.sync.dma_start(out=tl[:], in_=d_rw1[kc * P:(kc + 1) * P, :])
                rw1_t.append(tl)
            rw2_t = []
            for kc in range(4):
                tl = sb.tile([P, R_OUT], BF16, tag=f"rw2_{kc}")
                nc.sync.dma_start(out=tl[:], in_=d_rw2[kc * P:(kc + 1) * P, :])
                rw2_t.append(tl)
            uw1_t = sb.tile([UIN, UH], BF16)
            nc.sync.dma_start(out=uw1_t[:], in_=d_uw1[:])
            uw2a = sb.tile([P, U_OUT], BF16)
            nc.sync.dma_start(out=uw2a[:], in_=d_uw2[0:P, :])
            uw2b = sb.tile([UH - P, U_OUT], BF16)
            nc.sync.dma_start(out=uw2b[:], in_=d_uw2[P:UH, :])
            mlpb_t = sb.tile([P, 9], F32)
            nc.sync.dma_start(out=mlpb_t[:], in_=d_mlpb[:])
            b2bc_t = sb.tile([P, E], F32)
            nc.sync.dma_start(out=b2bc_t[:], in_=d_b2bc[:])
            if bp["exp_b1"]:
                b1p_t = sb.tile([EPC, EH], BF16)
                nc.sync.dma_start(out=b1p_t[:], in_=d_b1p[:])
                ones1 = sb.tile([1, P], BF16)
                nc.gpsimd.memset(ones1[:], 1.0)
            identb = sb.tile([P, P], BF16)
            make_identity(nc, identb[:])
            # expert weights: big, needed only in phase 2 -> emitted last
            w1e_t = []
            for e in range(EPC):
                tl = sb.tile([P, EIN // P, EH], BF16, tag=f"w1e{e}")
                for i in range(EIN // P):
                    nc.sync.dma_start(out=tl[:, i, :],
                                      in_=d_expw[e, i * P:(i + 1) * P, :])
                w1e_t.append(tl)

            zuT = sb.tile([P, P], BF16)
            zrT = sb.tile([P, R_OUT], BF16)

            with (
                tc.tile_pool(name="ptm", bufs=2, space="PSUM") as ptm,
                tc.tile_pool(name="pXG", bufs=1, space="PSUM") as pXG,
                tc.tile_pool(name="pG1", bufs=1, space="PSUM") as pG1,
            ):
                # ---- user MLP (independent of LSTM) ----
                tru = ptm.tile([P, P], BF16, tag="tm")
                nc.tensor.transpose(out=tru[:], in_=Uin[:], identity=identb[:])
                UinT = sb.tile([P, P], BF16)
                nc.vector.tensor_copy(out=UinT[:], in_=tru[:])
                u1ps = ptm.tile([P, 2 * P], F32, tag="tm")
                nc.tensor.matmul(out=u1ps[:, 0:P], lhsT=uw1_t[:, 0:P],
                                 rhs=UinT[0:UIN, :], start=True, stop=True)
                nc.tensor.matmul(out=u1ps[0:UH - P, P:2 * P], lhsT=uw1_t[:, P:UH],
                                 rhs=UinT[0:UIN, :], start=True, stop=True)
                U1T = sb.tile([P, 2 * P], BF16)
                nc.scalar.activation(U1T[:, 0:P], u1ps[:, 0:P], AF.Relu,
                                     bias=mlpb_t[:, 6:7])
                nc.scalar.activation(U1T[0:UH - P, P:2 * P], u1ps[0:UH - P, P:2 * P],
                                     AF.Relu, bias=mlpb_t[0:UH - P, 7:8])
                u2ps = ptm.tile([P, P], F32, tag="tm")
                nc.tensor.matmul(out=u2ps[:], lhsT=uw2a[:], rhs=U1T[:, 0:P],
                                 start=True, stop=False)
                nc.tensor.matmul(out=u2ps[:], lhsT=uw2b[:], rhs=U1T[0:UH - P, P:2 * P],
                                 start=False, stop=True)
                nc.scalar.activation(zuT[:], u2ps[:], AF.Identity,
                                     bias=mlpb_t[:, 8:9])

                zu_dr = dr.tile([P, P], BF16)
                nc.sync.dma_start(out=zu_dr[:], in_=zuT[:])
                Zall_u = dr.tile([NCORES, P, P], BF16, addr_space="Shared")
                if sim_single:
                    nc.sync.dma_start(out=Zall_u[0], in_=zu_dr[:])
                else:
                    nc.gpsimd.collective_compute(
                        "AllGather", ALU.bypass, ins=[zu_dr.opt()],
                        outs=[Zall_u.opt()],
                        replica_groups=[list(range(NCORES))])

                XT = sb.tile([P, T, RD], BF16)

                def transpose_x(t):
                    tr = ptm.tile([P, P], BF16, name="tr", tag="tm")
                    nc.tensor.transpose(out=tr[:], in_=X[:, t, :],
                                        identity=identb[:])
                    nc.vector.tensor_copy(out=XT[:, t, :], in_=tr[:])

                for t in range(4):
                    transpose_x(t)

                # ---- LSTM scan ----
                XG = [pXG.tile([P, 4 * RH], F32, name="XG0"),
                      pXG.tile([P, 4 * RH], F32, name="XG1")]
                G1 = pG1.tile([P, 4 * RH], F32, name="G1")
                # S holds sigmoid outputs [_, i, f, o] bf16; GC holds
                # [tanh_g | c] fp32 so ONE 512-wide DVE op forms
                # [t1|t2] = [i|f] * [g|c]; c stays fp32 (it accumulates
                # across the 50 steps)
                S = [sb.tile([P, 1024], BF16, name=f"S{l}") for l in range(2)]
                GC = [sb.tile([P, 512], F32, name=f"GC{l}") for l in range(2)]
                TT = [sb.tile([P, 512], F32, name=f"TT{l}") for l in range(2)]
                TC = [sb.tile([P, RH], BF16, name=f"TC{l}") for l in range(2)]
                # h0 double-buffered: layer-1 (delayed one step) still needs
                # h0(t-1) after cell(0,t) has produced h0(t)
                hT0 = [sb.tile([P, RH], BF16, name=f"hT0_{j}") for j in range(2)]
                hT1 = sb.tile([P, RH], BF16, name="hT1")


                def cell(l, t, g, h):
                    s, gc_, tc_ = S[l], GC[l], TC[l]
                    if bp["lstm_bias"]:
                        for jg in (0, 1):
                            nc.scalar.activation(gc_[:, jg * P:(jg + 1) * P],
                                                 g[:, jg * P:(jg + 1) * P],
                                                 AF.Tanh,
                                                 bias=lb_t[l][:, jg:jg + 1])
                        for jg in range(2, 8):
                            nc.scalar.activation(s[:, jg * P:(jg + 1) * P],
                                                 g[:, jg * P:(jg + 1) * P],
                                                 AF.Sigmoid,
                                                 bias=lb_t[l][:, jg:jg + 1])
                    elif l == 0:
                        # L0 is the critical chain: tanh(g) right after the
                        # first gate-pair group closes, sigmoid split
                        # (i,f | o) so the cell products start earlier
                        nc.scalar.activation(gc_[:, 0:256], g[:, 0:256], AF.Tanh)
                        nc.scalar.activation(s[:, 256:768], g[:, 256:768],
                                             AF.Sigmoid)
                        nc.scalar.activation(s[:, 768:1024], g[:, 768:1024],
                                             AF.Sigmoid)
                    else:
                        # L1 has a full step of slack: fewer, wider ACT calls
                        # keep the ACT queue clear for L0's chain
                        nc.scalar.activation(gc_[:, 0:256], g[:, 0:256], AF.Tanh)
                        nc.scalar.activation(s[:, 256:1024], g[:, 256:1024],
                                             AF.Sigmoid)
                    if t == 0:
                        nc.vector.tensor_tensor(out=gc_[:, 256:512],
                                                in0=s[:, 256:512],
                                                in1=gc_[:, 0:256], op=ALU.mult)
                    else:
                        # [t1|t2] = [i|f] * [g|c] in one 512-wide DVE op
                        nc.vector.tensor_tensor(out=TT[l][:, :],
                                                in0=s[:, 256:768],
                                                in1=gc_[:, :], op=ALU.mult)
                        nc.vector.tensor_tensor(out=gc_[:, 256:512],
                                                in0=TT[l][:, 0:256],
                                                in1=TT[l][:, 256:512], op=ALU.add)
                    nc.scalar.activation(tc_[:, :], gc_[:, 256:512], AF.Tanh)
                    nc.vector.tensor_tensor(out=h[:, :], in0=s[:, 768:1024],
                                            in1=tc_[:, :], op=ALU.mult)

                # PSUM group discipline: start=True clears has_written for the
                # WHOLE bank, so open each bank's group only on its first
                # slice. Groups CLOSE per gate-pair (jg 1,3,5,7) so tanh(g)
                # fires after only 4 recurrent matmuls.
                bank_first = lambda jg: jg % 4 == 0
                pair_last = lambda jg: jg % 2 == 1

                def l1_step(u):
                    """layer-1 MMs + cell for step u (issued one step late: at
                    issue time h0(u) is long ready -> no PE wait)."""
                    h0u = hT0[u % 2]
                    for jg in range(NG):
                        for kc in range(2):
                            nc.tensor.matmul(
                                out=G1[:, jg * P:(jg + 1) * P],
                                lhsT=wih_t[1][kc][:, jg * P:(jg + 1) * P],
                                rhs=h0u[:, kc * P:(kc + 1) * P],
                                start=(kc == 0 and bank_first(jg)),
                                stop=(u == 0 and kc == 1 and pair_last(jg)))
                    if u > 0:
                        for jg in range(NG):
                            for kc in range(2):
                                nc.tensor.matmul(
                                    out=G1[:, jg * P:(jg + 1) * P],
                                    lhsT=whh_t[1][kc][:, jg * P:(jg + 1) * P],
                                    rhs=hT1[:, kc * P:(kc + 1) * P],
                                    start=False, stop=(kc == 1 and pair_last(jg)))
                    cell(1, u, G1, hT1)

                # prologue: xg0 for t=0
                for jg in range(NG):
                    nc.tensor.matmul(out=XG[0][:, jg * P:(jg + 1) * P],
                                     lhsT=wih_t[0][0][:, jg * P:(jg + 1) * P],
                                     rhs=XT[:, 0, :], start=bank_first(jg),
                                     stop=pair_last(jg))

                for t in range(T):
                    Gx = XG[t % 2]
                    # L0 recurrent (the critical chain)
                    if t > 0:
                        for jg in range(NG):
                            for kc in range(2):
                                nc.tensor.matmul(
                                    out=Gx[:, jg * P:(jg + 1) * P],
                                    lhsT=whh_t[0][kc][:, jg * P:(jg + 1) * P],
                                    rhs=hT0[(t - 1) % 2][:, kc * P:(kc + 1) * P],
                                    start=False, stop=(kc == 1 and pair_last(jg)))
                    cell(0, t, Gx, hT0[t % 2])
                    # layer 1 for the previous step: all operands ready
                    if t > 0:
                        l1_step(t - 1)
                    # filler: xg0 for step t+1 (keeps PE warm, off-chain)
                    if t + 1 < T:
                        Gn = XG[(t + 1) % 2]
                        for jg in range(NG):
                            nc.tensor.matmul(
                                out=Gn[:, jg * P:(jg + 1) * P],
                                lhsT=wih_t[0][0][:, jg * P:(jg + 1) * P],
                                rhs=XT[:, t + 1, :], start=bank_first(jg), stop=False)
                    if t + 4 < T:
                        transpose_x(t + 4)
                l1_step(T - 1)

                # ---- ratings MLP ----
                r1ps = ptm.tile([P, RHID], F32, tag="tm")
                nmm = 0
                for mc in range(4):
                    for kc in range(2):
                        nmm += 1
                        nc.tensor.matmul(
                            out=r1ps[:, mc * P:(mc + 1) * P],
                            lhsT=rw1_t[kc][:, mc * P:(mc + 1) * P],
                            rhs=hT1[:, kc * P:(kc + 1) * P],
                            start=(nmm == 1), stop=(nmm == 8))
                R1T = sb.tile([P, RHID], BF16)
                if bp["mlp_bias"]:
                    for mc in range(4):
                        nc.scalar.activation(R1T[:, mc * P:(mc + 1) * P],
                                             r1ps[:, mc * P:(mc + 1) * P], AF.Relu,
                                             bias=mlpb_t[:, mc:mc + 1])
                else:
                    nc.scalar.activation(R1T[:], r1ps[:], AF.Relu)
                r2ps = ptm.tile([P, R_OUT], F32, tag="tm")
                nmm = 0
                for mc in range(2):
                    for kc in range(4):
                        nmm += 1
                        nc.tensor.matmul(
                            out=r2ps[:, mc * P:(mc + 1) * P],
                            lhsT=rw2_t[kc][:, mc * P:(mc + 1) * P],
                            rhs=R1T[:, kc * P:(kc + 1) * P],
                            start=(nmm == 1), stop=(nmm == 8))
                if bp["mlp_bias"]:
                    for mc in range(2):
                        nc.scalar.activation(zrT[:, mc * P:(mc + 1) * P],
                                             r2ps[:, mc * P:(mc + 1) * P],
                                             AF.Identity,
                                             bias=mlpb_t[:, 4 + mc:5 + mc])
                else:
                    nc.scalar.activation(zrT[:], r2ps[:], AF.Copy)

            # ---- allgather z (r half; the u half went out during p1) ----
            zr_dr = dr.tile([P, R_OUT], BF16)
            nc.sync.dma_start(out=zr_dr[:], in_=zrT[:])
            Zall_r = dr.tile([NCORES, P, R_OUT], BF16, addr_space="Shared")
            if sim_single:
                nc.sync.dma_start(out=Zall_r[0], in_=zr_dr[:])
            else:
                nc.gpsimd.collective_compute(
                    "AllGather", ALU.bypass, ins=[zr_dr.opt()],
                    outs=[Zall_r.opt()],
                    replica_groups=[list(range(NCORES))])

            # ---- experts ----
            with tc.tile_pool(name="phe", bufs=2, space="PSUM") as phe:
                crange = [0] if sim_single else list(range(NCORES))
                Zt = []
                for c in crange:
                    tl = sb.tile([P, EIN], BF16, tag=f"zt{c}")
                    nc.sync.dma_start(out=tl[:, 0:P], in_=Zall_u[c])
                    nc.sync.dma_start(out=tl[:, P:EIN], in_=Zall_r[c])
                    Zt.append(tl)
                scrd = sb.tile([P, EH], BF16)
                souts = sb.tile([P, E], F32)
                outs = sb.tile([P, E], F32)
                chunks = _chunks(EH)
                with tc.tile_pool(name="sw2", bufs=2) as sw2:
                    for e in range(EPC):
                        w2t = sw2.tile([P, EH], F32, name="w2t", tag="w2t")
                        nc.sync.dma_start(out=w2t[:], in_=d_w2r[e])
                        for c in crange:
                            ci = crange.index(c)
                            he = phe.tile([P, EH], F32, name="he", tag="he")
                            for i in range(EIN // P):
                                first = i == 0
                                last = (i == EIN // P - 1) and not bp["exp_b1"]
                                for (n0, n1) in chunks:
                                    nc.tensor.matmul(
                                        out=he[:, n0:n1],
                                        lhsT=Zt[ci][:, i * P:(i + 1) * P],
                                        rhs=w1e_t[e][:, i, n0:n1],
                                        start=first, stop=last)
                            if bp["exp_b1"]:
                                for (n0, n1) in chunks:
                                    nc.tensor.matmul(
                                        out=he[:, n0:n1], lhsT=ones1[:],
                                        rhs=b1p_t[e:e + 1, n0:n1],
                                        start=False, stop=True)
                            col = c * EPC + e
                            nc.vector.scalar_tensor_tensor(
                                out=scrd[:], in0=he[:], scalar=0.0,
                                in1=w2t[:], op0=ALU.max, op1=ALU.mult,
                                accum_out=souts[:, col:col + 1])
                n_used = len(crange) * EPC
                if bp["exp_b2"]:
                    nc.vector.tensor_tensor(out=outs[:, 0:n_used],
                                            in0=souts[:, 0:n_used],
                                            in1=b2bc_t[:, 0:n_used], op=ALU.add)
                    fin = outs
                else:
                    fin = souts
                for c in crange:
                    nc.sync.dma_start(out=d_out[c * P:(c + 1) * P, :],
                                      in_=fin[:, c * EPC:(c + 1) * EPC])
    nc.finalize()
    return nc


def _get_nc(bp, sim_single=False):
    key = (bp["lstm_bias"], bp["mlp_bias"], bp["exp_b1"], bp["exp_b2"],
           sim_single)
    if key not in _cache:
        _cache[key] = _build(bp, sim_single=sim_single)
    return _cache[key]


def run(inputs, trace=False):
    in_maps, bp = _prep(inputs)
    nc = _get_nc(bp)
    res = run_bass_kernel_spmd(nc, in_maps, core_ids=list(range(NCORES)),
                               trace=trace)
    out = np.concatenate([np.asarray(res.results[c]["out"]) for c in range(NCORES)],
                         axis=1).astype(np.float32)
    return out, res


def kernel(**inputs) -> np.ndarray:
    out, _ = run(inputs, trace=False)
    return out


# revision 28
# speedup vs baseline: 1.1505x; 1.1505x over previous
"""Trainium2 Bass kernel for nn_Recommender_2 (moe_routing).

Pipeline per core (8 NeuronCores, one TRN2 chip):
  Phase 1 (data-parallel over batch, 128 rows/core):
    - indirect-DMA gather of ratings embeddings (table pre-cast to bf16)
    - DMA-xbar transposes to time-major x^T tiles (no PE/DVE involvement)
    - 2-layer LSTM scan (T=50) in transposed [gate, batch] layout:
      weights stationary (lhsT), state h^T flows as rhs -> no per-step
      transposes. Gate order host-permuted to (g,i,f,o) so the cell needs
      only 3 ACT calls: tanh(g) after PSUM bank0, sigmoid(i|f|o) as one
      768-wide call, tanh(c). The two cell products t1=i*g / t2=f*c run
      on DVE and GpSimd in parallel (GpSimd once the embedding gathers
      have drained its queue). Layer-0 input MMs pipelined one step ahead
      into dedicated PSUM tiles; layer-1 runs one step delayed.
    - ratings MLP + user MLP (transposed layout) -> z^T [384,128] bf16
  Exchange: AllGather of z^T across the 8 cores -> Z^T [384,1024]
  Phase 2 (expert-parallel, 8 experts/core over full batch):
    - he = z @ W1_e accumulated in PSUM (bf16 weights)
    - the second GEMM (he @ w2_e, blocked by the relu) is eliminated; the
      reduction rotates over three engines by local slot (mod 3):
      ACT -> |w2| folded into W1, hidden units sign-partitioned
      [pos | pad | neg | pad] (unaligned boundary), two Relu accum_out
      calls (s+ - s-); DVE / GpSimd -> native W1, one
      scalar_tensor_tensor (he max 0)*w2 with accum_out.
Host reassembles [1024, 64] from per-core [1024, 8] outputs.
"""
import numpy as np
import ml_dtypes

import concourse.bacc as bacc
import concourse.bass as bass
import concourse.mybir as mybir
import concourse.tile as tile
from concourse.bass_utils import run_bass_kernel_spmd
from concourse.masks import make_identity

P = 128
NCORES = 8
B, T = 1024, 50
RV, RD, RH = 100000, 128, 256
R_OUT = 256
UV, UD, UDATA, U_OUT = 50000, 64, 32, 128
E, EIN, EH = 64, 384, 1536
EPC = E // NCORES
BL = B // NCORES
UIN = UD + UDATA
UH = 192
RHID = 512
NG = 8

F32 = mybir.dt.float32
BF16 = mybir.dt.bfloat16
FP8 = mybir.dt.float8e4
I32 = mybir.dt.int32
AF = mybir.ActivationFunctionType
ALU = mybir.AluOpType

# All experts reduce on DVE: relu(he)*w2 with accum_out, native hidden
# order (no sign-partition padding -> H2 == EH exactly). ACT stays free
# in phase 2; PE streaming is the binding engine.
N_W2 = EPC

_cache = {}


def _prep(inputs):
    f = lambda k: np.asarray(inputs[k], dtype=np.float32)
    bf = lambda a: np.ascontiguousarray(a, dtype=ml_dtypes.bfloat16)
    e4 = lambda a: np.ascontiguousarray(a, dtype=ml_dtypes.float8_e4m3)
    f32c = lambda a: np.ascontiguousarray(a, dtype=np.float32)

    # gate permutation i,f,g,o -> g,i,f,o  (bank0: g,i -> tanh(g) early,
    # then one sigmoid over i|f|o)
    perm = np.r_[512:768, 0:256, 256:512, 768:1024]

    shared = {}
    lstm_bias = False
    for l in range(2):
        wih = f(f"lstm_W_ih_{l}")[perm]
        whh = f(f"lstm_W_hh_{l}")[perm]
        bias = (f(f"lstm_b_ih_{l}") + f(f"lstm_b_hh_{l}"))[perm]
        shared[f"wih{l}"] = bf(wih.T)            # [in, 1024]
        shared[f"whh{l}"] = bf(whh.T)            # [256, 1024]
        shared[f"lb{l}"] = f32c(bias.reshape(NG, P).T)
        lstm_bias = lstm_bias or bool(np.any(bias))

    shared["rw1"] = bf(f("r_W1"))
    shared["rw2"] = bf(f("r_W2"))
    shared["uw1"] = bf(f("u_W1"))
    shared["uw2"] = bf(f("u_W2"))
    mlpb = np.zeros((P, 9), np.float32)
    mlpb[:, 0:4] = f("r_b1").reshape(4, P).T
    mlpb[:, 4:6] = f("r_b2").reshape(2, P).T
    ub1 = f("u_b1")
    mlpb[:, 6] = ub1[0:128]
    mlpb[0:64, 7] = ub1[128:192]
    mlpb[:, 8] = f("u_b2")
    mlp_bias = bool(np.any(mlpb))
    shared["mlpb"] = mlpb

    shared["remb"] = bf(f("ratings_emb"))        # bf16 table
    shared["uemb"] = bf(f("user_emb"))           # bf16 table

    w1 = f("exp_W1")                             # [64, 384, 1536]
    w2 = f("exp_W2").reshape(E, EH)              # [64, 1536]
    b1 = f("exp_b1")                             # [64, 1536]
    b2 = f("exp_b2").reshape(E)
    exp_b1_nz = bool(np.any(b1))

    ridx = np.asarray(inputs["ratings_tensor"]).astype(np.int32)
    uids = np.asarray(inputs["user_ids"]).astype(np.int32)
    udata = f("user_data")

    in_maps = []
    for c in range(NCORES):
        m = dict(shared)
        sl = slice(c * BL, (c + 1) * BL)
        m["ridx"] = np.ascontiguousarray(ridx[sl])
        m["uid"] = np.ascontiguousarray(uids[sl].reshape(BL, 1))
        m["udata"] = bf(udata[sl])
        es = slice(c * EPC, (c + 1) * EPC)
        m["expw"] = bf(w1[es])                   # [8, 384, 1536]
        m["w2r"] = f32c(np.broadcast_to(
            w2[es][:, None, :], (N_W2, P, EH)))
        m["b1p"] = bf(b1[es])                    # [8, 1536]
        b2bc = np.zeros((P, E), np.float32)
        for cc in range(NCORES):
            for e in range(EPC):
                b2bc[:, cc * EPC + e] = b2[c * EPC + e]
        m["b2bc"] = b2bc
        in_maps.append(m)

    bp = dict(lstm_bias=lstm_bias, mlp_bias=mlp_bias, exp_b1=exp_b1_nz,
              exp_b2=bool(np.any(b2)))
    return in_maps, bp


def _chunks(width):
    return [(s, min(s + 512, width)) for s in range(0, width, 512)]


def _build(bp, sim_single=False):

    nc = bacc.Bacc("TRN2", target_bir_lowering=False)
    d_ridx = nc.dram_tensor("ridx", [BL, T], I32, kind="ExternalInput")
    d_uid = nc.dram_tensor("uid", [BL, 1], I32, kind="ExternalInput")
    d_udata = nc.dram_tensor("udata", [BL, UDATA], BF16, kind="ExternalInput")
    d_remb = nc.dram_tensor("remb", [RV, RD], BF16, kind="ExternalInput")
    d_uemb = nc.dram_tensor("uemb", [UV, UD], BF16, kind="ExternalInput")
    d_wih = [nc.dram_tensor(f"wih{l}", [RD if l == 0 else RH, 4 * RH], BF16,
                            kind="ExternalInput") for l in range(2)]
    d_whh = [nc.dram_tensor(f"whh{l}", [RH, 4 * RH], BF16, kind="ExternalInput")
             for l in range(2)]
    d_lb = [nc.dram_tensor(f"lb{l}", [P, NG], F32, kind="ExternalInput")
            for l in range(2)]
    d_rw1 = nc.dram_tensor("rw1", [RH, RHID], BF16, kind="ExternalInput")
    d_rw2 = nc.dram_tensor("rw2", [RHID, R_OUT], BF16, kind="ExternalInput")
    d_uw1 = nc.dram_tensor("uw1", [UIN, UH], BF16, kind="ExternalInput")
    d_uw2 = nc.dram_tensor("uw2", [UH, U_OUT], BF16, kind="ExternalInput")
    d_mlpb = nc.dram_tensor("mlpb", [P, 9], F32, kind="ExternalInput")
    d_expw = nc.dram_tensor("expw", [EPC, EIN, EH], BF16, kind="ExternalInput")
    d_w2r = nc.dram_tensor("w2r", [N_W2, P, EH], F32, kind="ExternalInput")
    d_b1p = nc.dram_tensor("b1p", [EPC, EH], BF16, kind="ExternalInput")
    d_b2bc = nc.dram_tensor("b2bc", [P, E], F32, kind="ExternalInput")
    d_out = nc.dram_tensor("out", [B, EPC], F32, kind="ExternalOutput")

    with tile.TileContext(nc) as tc:
        with (
            tc.tile_pool(name="sb", bufs=1) as sb,
            tc.tile_pool(name="dr", bufs=1, space="DRAM") as dr,
        ):
            # ---- latency-critical loads first (indices feed the gathers) ----
            ridx_t = sb.tile([BL, T], I32)
            nc.sync.dma_start(out=ridx_t[:], in_=d_ridx[:])
            uid_t = sb.tile([BL, 1], I32)
            nc.sync.dma_start(out=uid_t[:], in_=d_uid[:])
            Uin = sb.tile([P, P], BF16)
            nc.gpsimd.indirect_dma_start(
                out=Uin[:, 0:UD], out_offset=None, in_=d_uemb[:],
                in_offset=bass.IndirectOffsetOnAxis(ap=uid_t[:, 0:1], axis=0))
            nc.sync.dma_start(out=Uin[:, UD:UIN], in_=d_udata[:])
            X = sb.tile([P, T, RD], BF16)
            for t in range(T):
                nc.gpsimd.indirect_dma_start(
                    out=X[:, t, :], out_offset=None, in_=d_remb[:],
                    in_offset=bass.IndirectOffsetOnAxis(ap=ridx_t[:, t:t + 1], axis=0))

            # ---- small static weights ----
            wih_t = []
            whh_t = []
            for l in range(2):
                kin = RD if l == 0 else RH
                wt = []
                for kc in range(kin // P):
                    tl = sb.tile([P, 4 * RH], BF16, tag=f"wih{l}_{kc}")
                    nc.sync.dma_start(out=tl[:], in_=d_wih[l][kc * P:(kc + 1) * P, :])
                    wt.append(tl)
                wih_t.append(wt)
                ht = []
                for kc in range(2):
                    tl = sb.tile([P, 4 * RH], BF16, tag=f"whh{l}_{kc}")
                    nc.sync.dma_start(out=tl[:], in_=d_whh[l][kc * P:(kc + 1) * P, :])
                    ht.append(tl)
                whh_t.append(ht)
            lb_t = []
            for l in range(2):
                tl = sb.tile([P, NG], F32, tag=f"lb{l}")
                nc.sync.dma_start(out=tl[:], in_=d_lb[l][:])
                lb_t.append(tl)
            rw1_t = []
            for kc in range(2):
                tl = sb.tile([P, RHID], BF16, tag=f"rw1_{kc}")
                nc.sync.dma_start(out=tl[:], in_=d_rw1[kc * P:(kc + 1) * P, :])
                rw1_t.append(tl)
            rw2_t = []
            for kc in range(4):
                tl = sb.tile([P, R_OUT], BF16, tag=f"rw2_{kc}")
                nc.sync.dma_start(out=tl[:], in_=d_rw2[kc * P:(kc + 1) * P, :])
                rw2_t.append(tl)
            uw1_t = sb.tile([UIN, UH], BF16)
            nc.sync.dma_start(out=uw1_t[:], in_=d_uw1[:])
            uw2a = sb.tile([P, U_OUT], BF16)
            nc.sync.dma_start(out=uw2a[:], in_=d_uw2[0:P, :])
            uw2b = sb.tile([UH - P, U_OUT], BF16)
            nc.sync.dma_start(out=uw2b[:], in_=d_uw2[P:UH, :])
            mlpb_t = sb.tile([P, 9], F32)
            nc.sync.dma_start(out=mlpb_t[:], in_=d_mlpb[:])
            b2bc_t = sb.tile([P, E], F32)
            nc.sync.dma_start(out=b2bc_t[:], in_=d_b2bc[:])
            if bp["exp_b1"]:
                b1p_t = sb.tile([EPC, EH], BF16)
                nc.sync.dma_start(out=b1p_t[:], in_=d_b1p[:])
                ones1 = sb.tile([1, P], BF16)
                nc.gpsimd.memset(ones1[:], 1.0)
            identb = sb.tile([P, P], BF16)
            make_identity(nc, identb[:])
            # expert weights: big, needed only in phase 2 -> emitted last
            w1e_t = []
            for e in range(EPC):
                tl = sb.tile([P, EIN // P, EH], BF16, tag=f"w1e{e}")
                for i in range(EIN // P):
                    nc.sync.dma_start(out=tl[:, i, :],
                                      in_=d_expw[e, i * P:(i + 1) * P, :])
                w1e_t.append(tl)

            zuT = sb.tile([P, P], BF16)
            zrT = sb.tile([P, R_OUT], BF16)

            with (
                tc.tile_pool(name="ptm", bufs=1, space="PSUM") as ptm,
                tc.tile_pool(name="pwm", bufs=1, space="PSUM") as pwm,
                tc.tile_pool(name="pXG", bufs=1, space="PSUM") as pXG,
                tc.tile_pool(name="pG1", bufs=1, space="PSUM") as pG1,
            ):
                # ---- user MLP (independent of LSTM) ----
                tru = ptm.tile([P, P], BF16, tag="tm")
                nc.tensor.transpose(out=tru[:], in_=Uin[:], identity=identb[:])
                UinT = sb.tile([P, P], BF16)
                nc.vector.tensor_copy(out=UinT[:], in_=tru[:])
                u1ps = ptm.tile([P, 2 * P], F32, tag="tm")
                nc.tensor.matmul(out=u1ps[:, 0:P], lhsT=uw1_t[:, 0:P],
                                 rhs=UinT[0:UIN, :], start=True, stop=True)
                nc.tensor.matmul(out=u1ps[0:UH - P, P:2 * P], lhsT=uw1_t[:, P:UH],
                                 rhs=UinT[0:UIN, :], start=True, stop=True)
                U1T = sb.tile([P, 2 * P], BF16)
                nc.scalar.activation(U1T[:, 0:P], u1ps[:, 0:P], AF.Relu,
                                     bias=mlpb_t[:, 6:7])
                nc.scalar.activation(U1T[0:UH - P, P:2 * P], u1ps[0:UH - P, P:2 * P],
                                     AF.Relu, bias=mlpb_t[0:UH - P, 7:8])
                u2ps = ptm.tile([P, P], F32, tag="tm")
                nc.tensor.matmul(out=u2ps[:], lhsT=uw2a[:], rhs=U1T[:, 0:P],
                                 start=True, stop=False)
                nc.tensor.matmul(out=u2ps[:], lhsT=uw2b[:], rhs=U1T[0:UH - P, P:2 * P],
                                 start=False, stop=True)
                nc.scalar.activation(zuT[:], u2ps[:], AF.Identity,
                                     bias=mlpb_t[:, 8:9])

                zu_dr = dr.tile([P, P], BF16)
                nc.sync.dma_start(out=zu_dr[:], in_=zuT[:])
                Zall_u = dr.tile([NCORES, P, P], BF16, addr_space="Shared")
                if sim_single:
                    nc.sync.dma_start(out=Zall_u[0], in_=zu_dr[:])
                else:
                    nc.gpsimd.collective_compute(
                        "AllGather", ALU.bypass, ins=[zu_dr.opt()],
                        outs=[Zall_u.opt()],
                        replica_groups=[list(range(NCORES))])

                XT = sb.tile([P, T, RD], BF16)

                def transpose_x(t):
                    tr = ptm.tile([P, P], BF16, name="tr", tag="tm")
                    nc.tensor.transpose(out=tr[:], in_=X[:, t, :],
                                        identity=identb[:])
                    nc.vector.tensor_copy(out=XT[:, t, :], in_=tr[:])

                for t in range(4):
                    transpose_x(t)

                # ---- LSTM scan ----
                XG = [pXG.tile([P, 4 * RH], F32, name="XG0"),
                      pXG.tile([P, 4 * RH], F32, name="XG1")]
                G1 = pG1.tile([P, 4 * RH], F32, name="G1")
                # S layout: [tanh_g | sig_i | sig_f | sig_o], 256 each, bf16
                # (pure-bf16 SBUF operands unlock the DVE 4x mode); c stays
                # fp32 (it accumulates across the 50 steps)
                S = [sb.tile([P, 1024], BF16, name=f"S{l}") for l in range(2)]
                C = [sb.tile([P, RH], F32, name=f"C{l}") for l in range(2)]
                TT1 = [sb.tile([P, RH], BF16, name=f"TT1{l}") for l in range(2)]
                TT2 = [sb.tile([P, RH], F32, name=f"TT2{l}") for l in range(2)]
                TC = [sb.tile([P, RH], BF16, name=f"TC{l}") for l in range(2)]
                # h0 double-buffered: layer-1 (delayed one step) still needs
                # h0(t-1) after cell(0,t) has produced h0(t)
                hT0 = [sb.tile([P, RH], BF16, name=f"hT0_{j}") for j in range(2)]
                hT1 = sb.tile([P, RH], BF16, name="hT1")
                warm = pwm.tile([P, 512], F32, name="warm")

                def keep_warm(n):
                    # dependency-free matmuls into a dead PSUM tile keep the
                    # PE streaming while it waits for h (an idle PE drops its
                    # p-state clock, slowing the next recurrent matmuls)
                    for _ in range(n):
                        nc.tensor.matmul(out=warm[:], lhsT=XT[:, 0, :],
                                         rhs=wih_t[0][0][:, 0:512],
                                         start=True, stop=True)


                def cell(l, t, g, h):
                    s, c_, tc_ = S[l], C[l], TC[l]
                    if bp["lstm_bias"]:
                        for jg in (0, 1):
                            nc.scalar.activation(s[:, jg * P:(jg + 1) * P],
                                                 g[:, jg * P:(jg + 1) * P],
                                                 AF.Tanh,
                                                 bias=lb_t[l][:, jg:jg + 1])
                        for jg in range(2, 8):
                            nc.scalar.activation(s[:, jg * P:(jg + 1) * P],
                                                 g[:, jg * P:(jg + 1) * P],
                                                 AF.Sigmoid,
                                                 bias=lb_t[l][:, jg:jg + 1])
                    else:
                        # tanh(g) after the first gate-pair group closes;
                        # sigmoid split (i,f | o) so t2 = f*c starts before
                        # sigmoid(o) occupies the ACT engine
                        nc.scalar.activation(s[:, 0:256], g[:, 0:256], AF.Tanh)
                        nc.scalar.activation(s[:, 256:768], g[:, 256:768],
                                             AF.Sigmoid)
                        nc.scalar.activation(s[:, 768:1024], g[:, 768:1024],
                                             AF.Sigmoid)
                    if t == 0:
                        nc.vector.tensor_tensor(out=c_[:, :], in0=s[:, 256:512],
                                                in1=s[:, 0:256], op=ALU.mult)
                    else:
                        # t2 = sig_f * c is the chain tail -> issue first
                        nc.vector.tensor_tensor(out=TT2[l][:, :],
                                                in0=s[:, 512:768],
                                                in1=c_[:, :], op=ALU.mult)
                        nc.vector.tensor_tensor(out=TT1[l][:, :],
                                                in0=s[:, 256:512],
                                                in1=s[:, 0:256], op=ALU.mult)
                        nc.vector.tensor_tensor(out=c_[:, :], in0=TT1[l][:, :],
                                                in1=TT2[l][:, :], op=ALU.add)
                    nc.scalar.activation(tc_[:, :], c_[:, :], AF.Tanh)
                    nc.vector.tensor_tensor(out=h[:, :], in0=s[:, 768:1024],
                                            in1=tc_[:, :], op=ALU.mult)

                # PSUM group discipline: start=True clears has_written for the
                # WHOLE bank, so open each bank's group only on its first
                # slice. Groups CLOSE per gate-pair (jg 1,3,5,7) so tanh(g)
                # fires after only 4 recurrent matmuls.
                bank_first = lambda jg: jg % 4 == 0
                pair_last = lambda jg: jg % 2 == 1

                def l1_step(u):
                    """layer-1 MMs + cell for step u (issued one step late: at
                    issue time h0(u) is long ready -> no PE wait)."""
                    h0u = hT0[u % 2]
                    for jg in range(NG):
                        for kc in range(2):
                            nc.tensor.matmul(
                                out=G1[:, jg * P:(jg + 1) * P],
                                lhsT=wih_t[1][kc][:, jg * P:(jg + 1) * P],
                                rhs=h0u[:, kc * P:(kc + 1) * P],
                                start=(kc == 0 and bank_first(jg)),
                                stop=(u == 0 and kc == 1 and pair_last(jg)))
                    if u > 0:
                        for jg in range(NG):
                            for kc in range(2):
                                nc.tensor.matmul(
                                    out=G1[:, jg * P:(jg + 1) * P],
                                    lhsT=whh_t[1][kc][:, jg * P:(jg + 1) * P],
                                    rhs=hT1[:, kc * P:(kc + 1) * P],
                                    start=False, stop=(kc == 1 and pair_last(jg)))
                    cell(1, u, G1, hT1)

                # prologue: xg0 for t=0
                for jg in range(NG):
                    nc.tensor.matmul(out=XG[0][:, jg * P:(jg + 1) * P],
                                     lhsT=wih_t[0][0][:, jg * P:(jg + 1) * P],
                                     rhs=XT[:, 0, :], start=bank_first(jg),
                                     stop=pair_last(jg))

                for t in range(T):
                    Gx = XG[t % 2]
                    # L0 recurrent (the critical chain)
                    if t > 0:
                        for jg in range(NG):
                            for kc in range(2):
                                nc.tensor.matmul(
                                    out=Gx[:, jg * P:(jg + 1) * P],
                                    lhsT=whh_t[0][kc][:, jg * P:(jg + 1) * P],
                                    rhs=hT0[(t - 1) % 2][:, kc * P:(kc + 1) * P],
                                    start=False, stop=(kc == 1 and pair_last(jg)))
                    cell(0, t, Gx, hT0[t % 2])
                    # layer 1 for the previous step: all operands ready
                    if t > 0:
                        l1_step(t - 1)
                    # filler: xg0 for step t+1 (keeps PE warm, off-chain)
                    if t + 1 < T:
                        Gn = XG[(t + 1) % 2]
                        for jg in range(NG):
                            nc.tensor.matmul(
                                out=Gn[:, jg * P:(jg + 1) * P],
                                lhsT=wih_t[0][0][:, jg * P:(jg + 1) * P],
                                rhs=XT[:, t + 1, :], start=bank_first(jg), stop=False)
                    if t + 4 < T:
                        transpose_x(t + 4)
                    keep_warm(3)
                l1_step(T - 1)

                # ---- ratings MLP ----
                r1ps = ptm.tile([P, RHID], F32, tag="tm")
                nmm = 0
                for mc in range(4):
                    for kc in range(2):
                        nmm += 1
                        nc.tensor.matmul(
                            out=r1ps[:, mc * P:(mc + 1) * P],
                            lhsT=rw1_t[kc][:, mc * P:(mc + 1) * P],
                            rhs=hT1[:, kc * P:(kc + 1) * P],
                            start=(nmm == 1), stop=(nmm == 8))
                R1T = sb.tile([P, RHID], BF16)
                if bp["mlp_bias"]:
                    for mc in range(4):
                        nc.scalar.activation(R1T[:, mc * P:(mc + 1) * P],
                                             r1ps[:, mc * P:(mc + 1) * P], AF.Relu,
                                             bias=mlpb_t[:, mc:mc + 1])
                else:
                    nc.scalar.activation(R1T[:], r1ps[:], AF.Relu)
                r2ps = ptm.tile([P, R_OUT], F32, tag="tm")
                nmm = 0
                for mc in range(2):
                    for kc in range(4):
                        nmm += 1
                        nc.tensor.matmul(
                            out=r2ps[:, mc * P:(mc + 1) * P],
                            lhsT=rw2_t[kc][:, mc * P:(mc + 1) * P],
                            rhs=R1T[:, kc * P:(kc + 1) * P],
                            start=(nmm == 1), stop=(nmm == 8))
                if bp["mlp_bias"]:
                    for mc in range(2):
                        nc.scalar.activation(zrT[:, mc * P:(mc + 1) * P],
                                             r2ps[:, mc * P:(mc + 1) * P],
                                             AF.Identity,
                                             bias=mlpb_t[:, 4 + mc:5 + mc])
                else:
                    nc.scalar.activation(zrT[:], r2ps[:], AF.Copy)

            # ---- allgather z (r half; the u half went out during p1) ----
            zr_dr = dr.tile([P, R_OUT], BF16)
            nc.sync.dma_start(out=zr_dr[:], in_=zrT[:])
            Zall_r = dr.tile([NCORES, P, R_OUT], BF16, addr_space="Shared")
            if sim_single:
                nc.sync.dma_start(out=Zall_r[0], in_=zr_dr[:])
            else:
                nc.gpsimd.collective_compute(
                    "AllGather", ALU.bypass, ins=[zr_dr.opt()],
                    outs=[Zall_r.opt()],
                    replica_groups=[list(range(NCORES))])

            # ---- experts ----
            with tc.tile_pool(name="phe", bufs=2, space="PSUM") as phe:
                crange = [0] if sim_single else list(range(NCORES))
                Zt = []
                for c in crange:
                    tl = sb.tile([P, EIN], BF16, tag=f"zt{c}")
                    nc.sync.dma_start(out=tl[:, 0:P], in_=Zall_u[c])
                    nc.sync.dma_start(out=tl[:, P:EIN], in_=Zall_r[c])
                    Zt.append(tl)
                scrd = sb.tile([P, EH], BF16)
                souts = sb.tile([P, E], F32)
                outs = sb.tile([P, E], F32)
                chunks = _chunks(EH)
                with tc.tile_pool(name="sw2", bufs=2) as sw2:
                    for e in range(EPC):
                        w2t = sw2.tile([P, EH], F32, name="w2t", tag="w2t")
                        nc.sync.dma_start(out=w2t[:], in_=d_w2r[e])
                        for c in crange:
                            ci = crange.index(c)
                            he = phe.tile([P, EH], F32, name="he", tag="he")
                            for i in range(EIN // P):
                                first = i == 0
                                last = (i == EIN // P - 1) and not bp["exp_b1"]
                                for (n0, n1) in chunks:
                                    nc.tensor.matmul(
                                        out=he[:, n0:n1],
                                        lhsT=Zt[ci][:, i * P:(i + 1) * P],
                                        rhs=w1e_t[e][:, i, n0:n1],
                                        start=first, stop=last)
                            if bp["exp_b1"]:
                                for (n0, n1) in chunks:
                                    nc.tensor.matmul(
                                        out=he[:, n0:n1], lhsT=ones1[:],
                                        rhs=b1p_t[e:e + 1, n0:n1],
                                        start=False, stop=True)
                            col = c * EPC + e
                            nc.vector.scalar_tensor_tensor(
                                out=scrd[:], in0=he[:], scalar=0.0,
                                in1=w2t[:], op0=ALU.max, op1=ALU.mult,
                                accum_out=souts[:, col:col + 1])
                n_used = len(crange) * EPC
                if bp["exp_b2"]:
                    nc.vector.tensor_tensor(out=outs[:, 0:n_used],
                                            in0=souts[:, 0:n_used],
                                            in1=b2bc_t[:, 0:n_used], op=ALU.add)
                    fin = outs
                else:
                    fin = souts
                for c in crange:
                    nc.sync.dma_start(out=d_out[c * P:(c + 1) * P, :],
                                      in_=fin[:, c * EPC:(c + 1) * EPC])
    nc.finalize()
    return nc


def _get_nc(bp, sim_single=False):
    key = (bp["lstm_bias"], bp["mlp_bias"], bp["exp_b1"], bp["exp_b2"],
           sim_single)
    if key not in _cache:
        _cache[key] = _build(bp, sim_single=sim_single)
    return _cache[key]


def run(inputs, trace=False):
    in_maps, bp = _prep(inputs)
    nc = _get_nc(bp)
    res = run_bass_kernel_spmd(nc, in_maps, core_ids=list(range(NCORES)),
                               trace=trace)
    out = np.concatenate([np.asarray(res.results[c]["out"]) for c in range(NCORES)],
                         axis=1).astype(np.float32)
    return out, res


def kernel(**inputs) -> np.ndarray:
    out, _ = run(inputs, trace=False)
    return out
